# Initial kernel scaffold
#
"""Trainium2 Bass kernel for YOLO-style detection decode (nms_detection).

Computes, for input `output` (B=8, H=80, W=80, A*85=255):
  per (b, cell, anchor):  xy = (sigmoid(txy) + grid_off) * stride
                          wh = exp(twh) * anchor
                          bbox = [xy - wh/2, xy + wh/2]
                          p_c = sigmoid(cls_c) * sigmoid(obj)
  out (B, C*hw*A, 6) rows = [cid, score, x1, y1, x2, y2] where
  cid = c if p_c > 0.01 else -1, score = p_c if p_c > 0.01 else 0.

Sharding: pure data parallel over batch, one batch element per NeuronCore.

Per-core layout strategy (output is 37 MB/core -> write-bandwidth bound):
  - cells are processed in supertiles of 512 (4 subtiles of 128 = partition dim)
  - class scores are transposed (TensorE) to class-major (80 partitions) so the
    per-class output block (hw, A, 6) is DMA'd with 9 KB contiguous segments
  - bbox (class-independent) is broadcast to all 80 class partitions with
    one-hot "selector" matmuls (K=12) on the otherwise idle TensorE
  - exp(x) is computed as sigmoid(x)/sigmoid(-x) so the ScalarE activation
    table never leaves the sigmoid set (a table switch costs ~2.7us).
"""

import sys
import os
from contextlib import ExitStack

if "/opt/trn_rl_repo" not in sys.path:
    sys.path.insert(0, "/opt/trn_rl_repo")

import numpy as np

NUM_CLASSES = 80
NUM_ANCHOR = 3
NUM_PRED = 85
HW_CELLS = 6400
THRESH = 0.01
N_CORES = 8
ROW = 6 * NUM_ANCHOR  # f32 per cell per class in the output (18)

_CACHE = {}


def _build(stride_f: float, n_cells: int = HW_CELLS):
    import concourse.bass as bass
    import concourse.tile as tile
    from concourse import mybir

    f32 = mybir.dt.float32
    AF = mybir.ActivationFunctionType
    OP = mybir.AluOpType

    C = NUM_CLASSES
    A = NUM_ANCHOR

    nc = bass.Bass()
    x_d = nc.declare_dram_parameter("x", [n_cells, A * NUM_PRED], f32, isOutput=False)
    offs_d = nc.declare_dram_parameter("offs", [n_cells // 128, 128, 6], f32, isOutput=False)
    hanch_d = nc.declare_dram_parameter("hanch", [128, 24], f32, isOutput=False)
    ident_d = nc.declare_dram_parameter("ident", [128, 128], f32, isOutput=False)
    sel_d = nc.declare_dram_parameter("sel", [12, 12 * C], f32, isOutput=False)
    cp1_d = nc.declare_dram_parameter("cp1", [128, 1], f32, isOutput=False)
    out_d = nc.declare_dram_parameter("out", [C, n_cells * ROW], f32, isOutput=True)

    # supertile = up to 4 subtiles of 128 cells
    st_sizes = []
    left = n_cells
    while left > 0:
        take = min(512, left)
        assert take % 128 == 0
        st_sizes.append(take)
        left -= take

    with ExitStack() as ctx:
        tc = ctx.enter_context(tile.TileContext(nc))
        cpool = ctx.enter_context(tc.tile_pool(name="const", bufs=1))
        in_pool = ctx.enter_context(tc.tile_pool(name="inp", bufs=3))
        sig_pool = ctx.enter_context(tc.tile_pool(name="sig", bufs=2))
        sm_pool = ctx.enter_context(tc.tile_pool(name="small", bufs=2))
        s_pool = ctx.enter_context(tc.tile_pool(name="scls", bufs=2))
        m_pool = ctx.enter_context(tc.tile_pool(name="mask", bufs=2))
        stg_pool = ctx.enter_context(tc.tile_pool(name="stage", bufs=2))
        o_pool = ctx.enter_context(tc.tile_pool(name="outt", bufs=2))
        p_pool = ctx.enter_context(tc.tile_pool(name="ppsum", bufs=2, space="PSUM"))
        pb_pool = ctx.enter_context(tc.tile_pool(name="pbpsum", bufs=2, space="PSUM"))
        q_pool = ctx.enter_context(tc.tile_pool(name="qpsum", bufs=2, space="PSUM"))

        # ---- constants (loaded once) ----
        offs_sb = cpool.tile([128, (n_cells // 128) * 6], f32, tag="offs")
        nc.scalar.dma_start(
            out=offs_sb[:, :].rearrange("p (t j) -> p t j", j=6),
            in_=offs_d[:, :, :].rearrange("t p j -> p t j"),
        )
        hanch_sb = cpool.tile([128, 24], f32, tag="hanch")
        nc.scalar.dma_start(out=hanch_sb[:, :], in_=hanch_d[:, :])
        ident_sb = cpool.tile([128, 128], f32, tag="ident")
        nc.scalar.dma_start(out=ident_sb[:, :], in_=ident_d[:, :])
        sel_sb = cpool.tile([12, 12 * C], f32, tag="sel")
        nc.scalar.dma_start(out=sel_sb[:, :], in_=sel_d[:, :])
        cp1_sb = cpool.tile([128, 1], f32, tag="cp1")
        nc.scalar.dma_start(out=cp1_sb[:, :], in_=cp1_d[:, :])

        c0 = 0
        for st, ncell in enumerate(st_sizes):
            ns = ncell // 128  # subtiles
            t0 = c0 // 128

            # ---- load input supertile: [p, s, 255] ----
            in_t = in_pool.tile([128, ns * 255], f32, tag="in")
            nc.scalar.dma_start(
                out=in_t[:, :].rearrange("p (s c) -> p s c", c=255),
                in_=x_d[c0 : c0 + ncell, :].rearrange("(s p) c -> p s c", p=128),
            )

            # ---- cell-major transforms ----
            sig = sig_pool.tile([128, ns * 255], f32, tag="sig")
            nc.scalar.activation(sig[:, :], in_t[:, :], AF.Sigmoid)

            in_v = in_t[:, :].rearrange("p (s a c) -> p s a c", a=A, c=NUM_PRED)
            sig_v = sig[:, :].rearrange("p (s a c) -> p s a c", a=A, c=NUM_PRED)

            # exp(wh) = sigmoid(wh) / sigmoid(-wh)
            sgnw = sm_pool.tile([128, ns * 6], f32, tag="sgnw")
            nc.scalar.activation(
                sgnw[:, :].rearrange("p (s a k) -> p s a k", a=A, k=2),
                in_v[:, :, :, 2:4],
                AF.Sigmoid,
                scale=-1.0,
            )
            rec = sm_pool.tile([128, ns * 6], f32, tag="rec")
            nc.vector.reciprocal(rec[:, :], sgnw[:, :])
            t1 = sm_pool.tile([128, ns * 6], f32, tag="t1")
            nc.vector.tensor_tensor(
                t1[:, :].rearrange("p (s a k) -> p s a k", a=A, k=2),
                sig_v[:, :, :, 2:4],
                hanch_sb[:, : ns * 6].rearrange("p (s a k) -> p s a k", a=A, k=2),
                OP.mult,
            )
            halfwh = sm_pool.tile([128, ns * 6], f32, tag="halfwh")
            nc.vector.tensor_tensor(halfwh[:, :], t1[:, :], rec[:, :], OP.mult)

            # xy = sigmoid(xy)*stride + off*stride
            xy = sm_pool.tile([128, ns * 6], f32, tag="xy")
            nc.vector.scalar_tensor_tensor(
                xy[:, :].rearrange("p (s a k) -> p s a k", a=A, k=2),
                in0=sig_v[:, :, :, 0:2],
                scalar=stride_f,
                in1=offs_sb[:, t0 * 6 : (t0 + ns) * 6].rearrange(
                    "p (s a k) -> p s a k", a=A, k=2
                ),
                op0=OP.mult,
                op1=OP.add,
            )

            # bbox cell-major: [p, s, a, 4] = [x1 y1 x2 y2]
            bb = sm_pool.tile([128, ns * 12], f32, tag="bb")
            bb_v = bb[:, :].rearrange("p (s a k) -> p s a k", a=A, k=4)
            xy_v = xy[:, :].rearrange("p (s a k) -> p s a k", a=A, k=2)
            hw_v = halfwh[:, :].rearrange("p (s a k) -> p s a k", a=A, k=2)
            nc.vector.tensor_tensor(bb_v[:, :, :, 0:2], xy_v, hw_v, OP.subtract)
            nc.vector.tensor_tensor(bb_v[:, :, :, 2:4], xy_v, hw_v, OP.add)

            # class scores = sigmoid(cls) * sigmoid(obj), cell-major (gpsimd)
            S = s_pool.tile([128, ns * A * C], f32, tag="S")
            for s in range(ns):
                for a in range(A):
                    base = s * 255 + a * NUM_PRED
                    nc.gpsimd.tensor_scalar(
                        S[:, (s * A + a) * C : (s * A + a + 1) * C],
                        sig[:, base + 5 : base + 85],
                        sig[:, base + 4 : base + 5],
                        None,
                        OP.mult,
                    )

            # bbox transpose to [12, ncell] (a,k major)
            pb = pb_pool.tile([12, ncell], f32, tag="pb")
            for s in range(ns):
                nc.tensor.transpose(
                    pb[:, s * 128 : (s + 1) * 128],
                    bb[:, s * 12 : (s + 1) * 12],
                    ident_sb[:, :],
                )
            bbt = stg_pool.tile([12, ncell], f32, tag="bbt")
            nc.vector.tensor_copy(bbt[:, :], pb[:, :])

            # output supertile, class-major
            outt = o_pool.tile([C, ncell * ROW], f32, tag="outt")
            ov = outt[:, :].rearrange("c (i e) -> c e i", e=ROW)

            for a in range(A):
                # transpose scores of anchor a -> [C, ncell]
                P = p_pool.tile([C, ncell], f32, tag="P")
                for s in range(ns):
                    nc.tensor.transpose(
                        P[:, s * 128 : (s + 1) * 128],
                        S[:, (s * A + a) * C : (s * A + a + 1) * C],
                        ident_sb[:, :],
                    )
                mask = m_pool.tile([C, ncell], f32, tag="mask")
                nc.vector.tensor_scalar(mask[:, :], P[:, :], THRESH, None, OP.is_gt)
                # score -> column a*6+1 (strided 18)
                nc.vector.tensor_tensor(ov[:, a * 6 + 1, :], P[:, :], mask[:, :], OP.mult)
                # cid = mask*(c+1) - 1 -> column a*6+0
                nc.scalar.activation(
                    ov[:, a * 6 + 0, :],
                    mask[:, :],
                    AF.Copy,
                    bias=-1.0,
                    scale=cp1_sb[0:C, :],
                )

                # bbox broadcast via one-hot selector matmuls, 2 PSUM banks
                # per pair of bbox channels
                for half in range(2):
                    q = q_pool.tile([C, 2 * ncell], f32, tag="q")
                    for kk in range(2):
                        k = half * 2 + kk
                        j = a * 4 + k
                        nc.tensor.matmul(
                            q[:, kk * ncell : (kk + 1) * ncell],
                            lhsT=sel_sb[:, j * C : (j + 1) * C],
                            rhs=bbt[:, :],
                            start=True,
                            stop=True,
                        )
                    # drain both channels in one strided op
                    dst = ov[:, a * 6 + 2 + half * 2 : a * 6 + 4 + half * 2, :]
                    src = q[:, :].rearrange("c (k i) -> c k i", k=2)
                    if (a * 2 + half) % 2 == 0:
                        nc.scalar.copy(dst, src)
                    else:
                        nc.vector.tensor_copy(dst, src)

            # ---- store ----
            nc.sync.dma_start(
                out=out_d[:, c0 * ROW : (c0 + ncell) * ROW], in_=outt[:, :]
            )
            c0 += ncell

    return nc


def _host_prep(output, anchor, offset, stride):
    stride_f = float(stride)
    B = output.shape[0]
    x_all = np.ascontiguousarray(
        np.asarray(output, dtype=np.float32).reshape(B, HW_CELLS, NUM_ANCHOR * NUM_PRED)
    )
    off = np.asarray(offset, dtype=np.float32).reshape(HW_CELLS, 2) * stride_f
    offs6 = np.ascontiguousarray(
        np.tile(off, (1, 3)).reshape(HW_CELLS // 128, 128, 6).astype(np.float32)
    )
    a2 = np.asarray(anchor, dtype=np.float32).reshape(NUM_ANCHOR, 2)
    hanch = np.ascontiguousarray(np.tile((a2 / 2.0).reshape(6), (128, 4)).astype(np.float32))
    ident = np.eye(128, dtype=np.float32)
    sel = np.zeros((12, 12, NUM_CLASSES), dtype=np.float32)
    for k in range(12):
        sel[k, k, :] = 1.0
    sel = np.ascontiguousarray(sel.reshape(12, 12 * NUM_CLASSES))
    cp1 = np.arange(1, 129, dtype=np.float32).reshape(128, 1)
    return stride_f, x_all, offs6, hanch, ident, sel, cp1


def kernel(output, anchor, offset, stride):
    from concourse.bass_utils import run_bass_kernel_spmd

    stride_f, x_all, offs6, hanch, ident, sel, cp1 = _host_prep(
        output, anchor, offset, stride
    )
    key = ("nc", stride_f)
    if key not in _CACHE:
        _CACHE[key] = _build(stride_f)
    nc = _CACHE[key]

    in_maps = [
        {
            "x": x_all[b],
            "offs": offs6,
            "hanch": hanch,
            "ident": ident,
            "sel": sel,
            "cp1": cp1,
        }
        for b in range(N_CORES)
    ]
    res = run_bass_kernel_spmd(nc, in_maps, list(range(N_CORES)))
    outs = [
        r["out"].reshape(NUM_CLASSES * HW_CELLS * NUM_ANCHOR, 6) for r in res.results
    ]
    return np.stack(outs, axis=0)


if __name__ == "__main__":
    rng = np.random.default_rng(0)
    out = rng.standard_normal((8, 80, 80, 255), dtype=np.float32)
    anchor = rng.uniform(10.0, 120.0, (1, 1, 3, 2)).astype(np.float32)
    gy, gx = np.meshgrid(np.arange(80, dtype=np.float32), np.arange(80, dtype=np.float32), indexing="ij")
    offset = np.stack([gx, gy], axis=-1).reshape(1, 80, 80, 1, 2)
    r = kernel(out, anchor, offset, 8)
    print(r.shape, r.dtype)


# revision 43
# speedup vs baseline: 1.7672x; 1.7672x over previous
"""Trainium2 Bass kernel for YOLO-style detection decode (nms_detection).

Computes, for input `output` (B=8, H=80, W=80, A*85=255):
  per (b, cell, anchor):  xy = (sigmoid(txy) + grid_off) * stride
                          wh = exp(twh) * anchor
                          bbox = [xy - wh/2, xy + wh/2]
                          p_c = sigmoid(cls_c) * sigmoid(obj)
  out (B, C*hw*A, 6) rows = [cid, score, x1, y1, x2, y2] where
  cid = c if p_c > 0.01 else -1, score = p_c if p_c > 0.01 else 0.

Sharding: pure data parallel over batch, one batch element per NeuronCore.

Per-core layout strategy (output is 37 MB/core -> write-bandwidth bound):
  - cells are processed in supertiles of 512 (4 subtiles of 128 = partition dim)
  - class scores are transposed (TensorE) to class-major (80 partitions) so the
    per-class output block (hw, A, 6) is DMA'd with 9 KB contiguous segments
  - bbox (class-independent) is broadcast to all 80 class partitions with
    one-hot "selector" matmuls (K=12) on the otherwise idle TensorE
  - exp(x) is computed as sigmoid(x)/sigmoid(-x) so the ScalarE activation
    table never leaves the sigmoid set (a table switch costs ~2.7us).
"""

import sys
import os
from contextlib import ExitStack

if "/opt/trn_rl_repo" not in sys.path:
    sys.path.insert(0, "/opt/trn_rl_repo")

import numpy as np

NUM_CLASSES = 80
NUM_ANCHOR = 3
NUM_PRED = 85
HW_CELLS = 6400
THRESH = 0.01
N_CORES = 8
ROW = 6 * NUM_ANCHOR  # f32 per cell per class in the output (18)

_CACHE = {}
LAST_RESULT = None  # BassKernelResults of the most recent kernel() call


# Row groups used for the broadcast matmuls (1, 2, or 4). Multi-group runs
# the per-anchor broadcasts concurrently on distinct PE row groups.
ROW_GROUPS = int(os.environ.get("KERNEL_ROW_GROUPS", "1"))
BASES = (96, 0, 32, 64)
BCAST_DTYPE = os.environ.get("KERNEL_BCAST_DTYPE", "f32r")  # f32r | f32


def _build(stride_f: float, n_cells: int = HW_CELLS):
    import concourse.bass as bass  # noqa: F401
    import concourse.bacc as bacc
    import concourse.tile as tile
    from concourse import mybir

    f32 = mybir.dt.float32
    f32r = mybir.dt.float32r
    AF = mybir.ActivationFunctionType
    OP = mybir.AluOpType

    C = NUM_CLASSES
    A = NUM_ANCHOR

    n_tiles = n_cells // 128
    CONST_F = n_tiles * 6 + 24 + 128 + 1 + 12 * C  # offs | hanch | ident | cp1 | sel
    OFF_HANCH = n_tiles * 6
    OFF_IDENT = OFF_HANCH + 24
    OFF_CP1 = OFF_IDENT + 128
    OFF_SEL = OFF_CP1 + 1

    nc = bacc.Bacc("TRN2", target_bir_lowering=False, debug=False)
    x_d = nc.declare_dram_parameter("x", [n_cells, A * NUM_PRED], f32, isOutput=False)
    const_d = nc.declare_dram_parameter("consts", [128, CONST_F], f32, isOutput=False)
    out_d = nc.declare_dram_parameter("out", [C, n_cells * ROW], f32, isOutput=True)

    # supertile = up to 4 subtiles of 128 cells
    st_sizes = []
    left = n_cells
    while left > 0:
        take = min(512, left)
        assert take % 128 == 0
        st_sizes.append(take)
        left -= take

    with ExitStack() as ctx:
        tc = ctx.enter_context(tile.TileContext(nc))
        cpool = ctx.enter_context(tc.tile_pool(name="const", bufs=1))
        in_pool = ctx.enter_context(tc.tile_pool(name="inp", bufs=3))
        sig_pool = ctx.enter_context(tc.tile_pool(name="sig", bufs=2))
        sm_pool = ctx.enter_context(tc.tile_pool(name="small", bufs=2))
        s_pool = ctx.enter_context(tc.tile_pool(name="scls", bufs=2))
        m_pool = ctx.enter_context(tc.tile_pool(name="mask", bufs=2))
        stg_pool = ctx.enter_context(tc.tile_pool(name="stage", bufs=2))
        o_pool = ctx.enter_context(tc.tile_pool(name="outt", bufs=2))
        p_pool = ctx.enter_context(tc.tile_pool(name="ppsum", bufs=3, space="PSUM"))
        q_pool = ctx.enter_context(tc.tile_pool(name="qpsum", bufs=2, space="PSUM"))

        # ---- constants (one DMA -> one sem lane) ----
        const_sb = cpool.tile([128, CONST_F], f32, tag="consts")
        nc.scalar.dma_start(out=const_sb[:, :], in_=const_d[:, :])
        offs_sb = const_sb[:, 0:OFF_HANCH]
        hanch_sb = const_sb[:, OFF_HANCH:OFF_IDENT]
        ident_sb = const_sb[:, OFF_IDENT:OFF_CP1]
        cp1_sb = const_sb[:, OFF_CP1:OFF_SEL]
        sel_sb = const_sb[:, OFF_SEL:CONST_F]
        bc_dt = f32r if BCAST_DTYPE == "f32r" else f32
        sel_r = cpool.tile([128, 12 * C], bc_dt, tag="selr")
        nc.vector.tensor_copy(sel_r[:, :], sel_sb)

        # ---- warm-up: let each engine observe the const DMA once, so no
        # later instruction needs more than one sync-wait (ISA limit) ----
        warm = cpool.tile([128, 4], f32, tag="warm")
        nc.vector.tensor_copy(warm[0:1, 0:1], const_sb[0:1, 0:1])
        nc.scalar.copy(warm[0:1, 1:2], const_sb[0:1, 0:1])
        nc.gpsimd.tensor_copy(warm[0:1, 2:3], const_sb[0:1, 0:1])
        wq = p_pool.tile([128, 128], f32, tag="P")
        nc.tensor.transpose(wq[:, :], ident_sb, ident_sb)

        c0 = 0
        for st, ncell in enumerate(st_sizes):
            ns = ncell // 128  # subtiles
            t0 = c0 // 128

            # ---- load input supertile: [p, s, 255] ----
            in_t = in_pool.tile([128, ns * 255], f32, tag="in")
            nc.scalar.dma_start(
                out=in_t[:, :].rearrange("p (s c) -> p s c", c=255),
                in_=x_d[c0 : c0 + ncell, :].rearrange("(s p) c -> p s c", p=128),
            )

            # ---- cell-major transforms ----
            sig = sig_pool.tile([128, ns * 255], f32, tag="sig")
            nc.scalar.activation(sig[:, :], in_t[:, :], AF.Sigmoid)

            in_v = in_t[:, :].rearrange("p (s a c) -> p s a c", a=A, c=NUM_PRED)
            sig_v = sig[:, :].rearrange("p (s a c) -> p s a c", a=A, c=NUM_PRED)

            # exp(wh) = sigmoid(wh) / sigmoid(-wh)
            sgnw = sm_pool.tile([128, ns * 6], f32, tag="sgnw")
            nc.scalar.activation(
                sgnw[:, :].rearrange("p (s a k) -> p s a k", a=A, k=2),
                in_v[:, :, :, 2:4],
                AF.Sigmoid,
                scale=-1.0,
            )
            rec = sm_pool.tile([128, ns * 6], f32, tag="rec")
            nc.vector.reciprocal(rec[:, :], sgnw[:, :])
            t1 = sm_pool.tile([128, ns * 6], f32, tag="t1")
            nc.vector.tensor_tensor(
                t1[:, :].rearrange("p (s a k) -> p s a k", a=A, k=2),
                sig_v[:, :, :, 2:4],
                hanch_sb[:, : ns * 6].rearrange("p (s a k) -> p s a k", a=A, k=2),
                OP.mult,
            )
            halfwh = sm_pool.tile([128, ns * 6], f32, tag="halfwh")
            nc.vector.tensor_tensor(halfwh[:, :], t1[:, :], rec[:, :], OP.mult)

            # xy = sigmoid(xy)*stride + off*stride
            xy = sm_pool.tile([128, ns * 6], f32, tag="xy")
            nc.vector.scalar_tensor_tensor(
                xy[:, :].rearrange("p (s a k) -> p s a k", a=A, k=2),
                in0=sig_v[:, :, :, 0:2],
                scalar=stride_f,
                in1=offs_sb[:, t0 * 6 : (t0 + ns) * 6].rearrange(
                    "p (s a k) -> p s a k", a=A, k=2
                ),
                op0=OP.mult,
                op1=OP.add,
            )

            # per-subtile block layout [S_a0 | S_a1 | S_a2 | pad 16 | bb 12] so
            # anchor 2's transpose carries the bbox columns for free, landing
            # them on PE row group 3 (partitions 96..107)
            SW = A * C + 16 + 12  # 268
            S = s_pool.tile([128, ns * SW], f32, tag="S")

            # bbox cell-major -> S cols [240:252) per subtile: [a, 4] = x1 y1 x2 y2
            S_v = S[:, :].rearrange("p (s w) -> p s w", w=SW)
            bb_v = S[:, :].rearrange("p (s w) -> p s w", w=SW)[
                :, :, A * C + 16 : SW
            ].rearrange("p s (a k) -> p s a k", k=4)
            xy_v = xy[:, :].rearrange("p (s a k) -> p s a k", a=A, k=2)
            hw_v = halfwh[:, :].rearrange("p (s a k) -> p s a k", a=A, k=2)
            nc.gpsimd.tensor_tensor(bb_v[:, :, :, 0:2], xy_v, hw_v, OP.subtract)
            nc.gpsimd.tensor_tensor(bb_v[:, :, :, 2:4], xy_v, hw_v, OP.add)

            # class scores = sigmoid(cls) * sigmoid(obj), cell-major; obj is
            # broadcast along the class dim with a stride-0 AP (gpsimd: DVE
            # and ACT are the busy engines)
            nc.gpsimd.tensor_tensor(
                S[:, :]
                .rearrange("p (s w) -> p s w", w=SW)[:, :, 0 : A * C]
                .rearrange("p s (a c) -> p s a c", c=C),
                sig_v[:, :, :, 5:85],
                sig_v[:, :, :, 4:5].to_broadcast([128, ns, A, C]),
                OP.mult,
            )

            # output supertile, class-major
            outt = o_pool.tile([C, ncell * ROW], f32, tag="outt")
            ov = outt[:, :].rearrange("c (i e) -> c e i", e=ROW)

            bbt = stg_pool.tile([128, ncell], bc_dt, tag="bbt")

            for a in (2, 0, 1):  # anchor 2 first: it stages the bbox rows
                # transpose scores of anchor a -> [C, ncell]; anchor 2 also
                # carries the 12 bbox rows into partitions 96..107
                pw = C + 28 if a == 2 else C
                P = p_pool.tile([C + 28, ncell], f32, tag="P")
                for s in range(ns):
                    nc.tensor.transpose(
                        P[0:pw, s * 128 : (s + 1) * 128],
                        S_v[:, s, a * C : a * C + pw],
                        ident_sb[:, :],
                    )
                if a == 2:
                    # stage bbox rows to SBUF (rounds to fp32r) on row group 3
                    nc.vector.tensor_copy(bbt[96:108, :], P[96:108, :])
                    for base in BASES[1:ROW_GROUPS]:
                        nc.sync.dma_start(
                            out=bbt[base : base + 12, :], in_=bbt[96:108, :]
                        )
                mask = m_pool.tile([C, ncell], f32, tag="mask")
                nc.vector.tensor_scalar(mask[:, :], P[0:C, :], THRESH, None, OP.is_gt)
                # score -> column a*6+1 (strided 18)
                nc.vector.tensor_tensor(
                    ov[:, a * 6 + 1, :], P[0:C, :], mask[:, :], OP.mult
                )
                # cid = mask*(c+1) - 1 -> column a*6+0 (gpsimd, SBUF-only op)
                nc.gpsimd.tensor_scalar(
                    ov[:, a * 6 + 0, :],
                    mask[:, :],
                    cp1_sb[0:C, :],
                    -1.0,
                    OP.mult,
                    OP.add,
                )

                # bbox broadcast via one-hot selector matmuls (fp32r, one
                # pass) on 4 distinct PE row groups -> concurrent
                for half in range(2):
                    q = q_pool.tile([C, 2 * ncell], f32, tag="q")
                    for kk in range(2):
                        k = half * 2 + kk
                        j = a * 4 + k
                        base = BASES[j % ROW_GROUPS]
                        nc.tensor.matmul(
                            q[:, kk * ncell : (kk + 1) * ncell],
                            lhsT=sel_r[base : base + 12, j * C : (j + 1) * C],
                            rhs=bbt[base : base + 12, :],
                            start=True,
                            stop=True,
                            tile_position=(base, 0),
                        )
                    # drain both channels in one strided op; ACT takes 4 of
                    # 6 (it is 1.25x faster per element and less loaded)
                    dst = ov[:, a * 6 + 2 + half * 2 : a * 6 + 4 + half * 2, :]
                    src = q[:, :].rearrange("c (k i) -> c k i", k=2)
                    if (a, half) in ((0, 1), (1, 1)):
                        nc.vector.tensor_copy(dst, src)
                    else:
                        nc.scalar.copy(dst, src)

            # ---- store ----
            nc.sync.dma_start(
                out=out_d[:, c0 * ROW : (c0 + ncell) * ROW], in_=outt[:, :]
            )
            c0 += ncell

    nc.finalize()
    return nc


def make_consts(anchor, offset, stride_f, n_cells=HW_CELLS):
    """Pack [offs | hanch | ident | cp1 | sel] into one (128, F) f32 blob."""
    n_tiles = n_cells // 128
    off = np.asarray(offset, dtype=np.float32).reshape(-1, 2)[:n_cells] * stride_f
    offs6 = np.tile(off, (1, 3)).reshape(n_tiles, 128, 6)  # [t, p, j]
    offs_cols = np.ascontiguousarray(np.transpose(offs6, (1, 0, 2)).reshape(128, n_tiles * 6))
    a2 = np.asarray(anchor, dtype=np.float32).reshape(NUM_ANCHOR, 2)
    hanch = np.tile((a2 / 2.0).reshape(6), (128, 4)).astype(np.float32)
    ident = np.eye(128, dtype=np.float32)
    cp1 = np.broadcast_to(np.arange(1, 129, dtype=np.float32).reshape(128, 1), (128, 1))
    # one-hot selector for bbox channel j, placed on PE row group j%4 so the
    # four per-anchor broadcast matmuls can row-tile concurrently
    # one-hot selectors for bbox channel j, for every row-group mapping the
    # kernel might use (distinct bases never collide within a column block)
    sel128 = np.zeros((128, 12 * NUM_CLASSES), dtype=np.float32)
    bases = (96, 0, 32, 64)
    for rg in (1, 2, 4):
        for j in range(12):
            sel128[bases[j % rg] + j, j * NUM_CLASSES : (j + 1) * NUM_CLASSES] = 1.0
    blob = np.concatenate([offs_cols, hanch, ident, cp1, sel128], axis=1)
    return np.ascontiguousarray(blob.astype(np.float32))


def _host_prep(output, anchor, offset, stride):
    stride_f = float(stride)
    B = output.shape[0]
    x_all = np.ascontiguousarray(
        np.asarray(output, dtype=np.float32).reshape(B, HW_CELLS, NUM_ANCHOR * NUM_PRED)
    )
    consts = make_consts(anchor, offset, stride_f)
    return stride_f, x_all, consts


def kernel(output, anchor, offset, stride):
    from concourse.bass_utils import run_bass_kernel_spmd

    stride_f, x_all, consts = _host_prep(output, anchor, offset, stride)
    key = ("nc", stride_f)
    if key not in _CACHE:
        _CACHE[key] = _build(stride_f)
    nc = _CACHE[key]

    in_maps = [{"x": x_all[b], "consts": consts} for b in range(N_CORES)]
    res = run_bass_kernel_spmd(
        nc,
        in_maps,
        list(range(N_CORES)),
        tmpdir=os.environ.get("KERNEL_TRACE_DIR") or None,
    )
    global LAST_RESULT
    LAST_RESULT = res
    outs = [
        r["out"].reshape(NUM_CLASSES * HW_CELLS * NUM_ANCHOR, 6) for r in res.results
    ]
    return np.stack(outs, axis=0)


if __name__ == "__main__":
    rng = np.random.default_rng(0)
    out = rng.standard_normal((8, 80, 80, 255), dtype=np.float32)
    anchor = rng.uniform(10.0, 120.0, (1, 1, 3, 2)).astype(np.float32)
    gy, gx = np.meshgrid(np.arange(80, dtype=np.float32), np.arange(80, dtype=np.float32), indexing="ij")
    offset = np.stack([gx, gy], axis=-1).reshape(1, 80, 80, 1, 2)
    r = kernel(out, anchor, offset, 8)
    print(r.shape, r.dtype)


# revision 44
# speedup vs baseline: 1.7929x; 1.0145x over previous
"""Trainium2 Bass kernel for YOLO-style detection decode (nms_detection).

Computes, for input `output` (B=8, H=80, W=80, A*85=255):
  per (b, cell, anchor):  xy = (sigmoid(txy) + grid_off) * stride
                          wh = exp(twh) * anchor
                          bbox = [xy - wh/2, xy + wh/2]
                          p_c = sigmoid(cls_c) * sigmoid(obj)
  out (B, C*hw*A, 6) rows = [cid, score, x1, y1, x2, y2] where
  cid = c if p_c > 0.01 else -1, score = p_c if p_c > 0.01 else 0.

Sharding: pure data parallel over batch, one batch element per NeuronCore.

Per-core layout strategy (output is 37 MB/core -> write-bandwidth bound):
  - cells are processed in supertiles of 512 (4 subtiles of 128 = partition dim)
  - class scores are transposed (TensorE) to class-major (80 partitions) so the
    per-class output block (hw, A, 6) is DMA'd with 9 KB contiguous segments
  - bbox (class-independent) is broadcast to all 80 class partitions with
    one-hot "selector" matmuls (K=12) on the otherwise idle TensorE
  - exp(x) is computed as sigmoid(x)/sigmoid(-x) so the ScalarE activation
    table never leaves the sigmoid set (a table switch costs ~2.7us).
"""

import sys
import os
from contextlib import ExitStack

if "/opt/trn_rl_repo" not in sys.path:
    sys.path.insert(0, "/opt/trn_rl_repo")

import numpy as np

NUM_CLASSES = 80
NUM_ANCHOR = 3
NUM_PRED = 85
HW_CELLS = 6400
THRESH = 0.01
N_CORES = 8
ROW = 6 * NUM_ANCHOR  # f32 per cell per class in the output (18)

_CACHE = {}
LAST_RESULT = None  # BassKernelResults of the most recent kernel() call


# Row groups used for the broadcast matmuls (1, 2, or 4). Multi-group runs
# the per-anchor broadcasts concurrently on distinct PE row groups.
ROW_GROUPS = int(os.environ.get("KERNEL_ROW_GROUPS", "1"))
BASES = (96, 0, 32, 64)
BCAST_DTYPE = os.environ.get("KERNEL_BCAST_DTYPE", "f32r")  # f32r | f32


def _build(stride_f: float, n_cells: int = HW_CELLS):
    import concourse.bass as bass  # noqa: F401
    import concourse.bacc as bacc
    import concourse.tile as tile
    from concourse import mybir

    f32 = mybir.dt.float32
    f32r = mybir.dt.float32r
    AF = mybir.ActivationFunctionType
    OP = mybir.AluOpType

    C = NUM_CLASSES
    A = NUM_ANCHOR

    n_tiles = n_cells // 128
    CONST_F = n_tiles * 6 + 24 + 128 + 1 + 12 * C  # offs | hanch | ident | cp1 | sel
    OFF_HANCH = n_tiles * 6
    OFF_IDENT = OFF_HANCH + 24
    OFF_CP1 = OFF_IDENT + 128
    OFF_SEL = OFF_CP1 + 1

    nc = bacc.Bacc("TRN2", target_bir_lowering=False, debug=False)
    x_d = nc.declare_dram_parameter("x", [n_cells, A * NUM_PRED], f32, isOutput=False)
    const_d = nc.declare_dram_parameter("consts", [128, CONST_F], f32, isOutput=False)
    out_d = nc.declare_dram_parameter("out", [C, n_cells * ROW], f32, isOutput=True)

    # supertile = up to 4 subtiles of 128 cells
    st_sizes = []
    left = n_cells
    while left > 0:
        take = min(512, left)
        assert take % 128 == 0
        st_sizes.append(take)
        left -= take

    with ExitStack() as ctx:
        tc = ctx.enter_context(tile.TileContext(nc))
        cpool = ctx.enter_context(tc.tile_pool(name="const", bufs=1))
        in_pool = ctx.enter_context(tc.tile_pool(name="inp", bufs=3))
        sig_pool = ctx.enter_context(tc.tile_pool(name="sig", bufs=2))
        sm_pool = ctx.enter_context(tc.tile_pool(name="small", bufs=2))
        s_pool = ctx.enter_context(tc.tile_pool(name="scls", bufs=2))
        m_pool = ctx.enter_context(tc.tile_pool(name="mask", bufs=2))
        stg_pool = ctx.enter_context(tc.tile_pool(name="stage", bufs=2))
        o_pool = ctx.enter_context(tc.tile_pool(name="outt", bufs=2))
        p_pool = ctx.enter_context(tc.tile_pool(name="ppsum", bufs=3, space="PSUM"))
        q_pool = ctx.enter_context(tc.tile_pool(name="qpsum", bufs=2, space="PSUM"))

        # ---- constants (one DMA -> one sem lane) ----
        const_sb = cpool.tile([128, CONST_F], f32, tag="consts")
        nc.scalar.dma_start(out=const_sb[:, :], in_=const_d[:, :])
        offs_sb = const_sb[:, 0:OFF_HANCH]
        hanch_sb = const_sb[:, OFF_HANCH:OFF_IDENT]
        ident_sb = const_sb[:, OFF_IDENT:OFF_CP1]
        cp1_sb = const_sb[:, OFF_CP1:OFF_SEL]
        sel_sb = const_sb[:, OFF_SEL:CONST_F]
        bc_dt = f32r if BCAST_DTYPE == "f32r" else f32
        sel_r = cpool.tile([128, 12 * C], bc_dt, tag="selr")
        nc.vector.tensor_copy(sel_r[:, :], sel_sb)

        # ---- warm-up: let each engine observe the const DMA once, so no
        # later instruction needs more than one sync-wait (ISA limit) ----
        warm = cpool.tile([128, 4], f32, tag="warm")
        nc.vector.tensor_copy(warm[0:1, 0:1], const_sb[0:1, 0:1])
        nc.scalar.copy(warm[0:1, 1:2], const_sb[0:1, 0:1])
        nc.gpsimd.tensor_copy(warm[0:1, 2:3], const_sb[0:1, 0:1])
        wq = p_pool.tile([128, 128], f32, tag="P")
        nc.tensor.transpose(wq[:, :], ident_sb, ident_sb)

        c0 = 0
        for st, ncell in enumerate(st_sizes):
            ns = ncell // 128  # subtiles
            t0 = c0 // 128

            # ---- load input supertile: [p, s, 255] ----
            in_t = in_pool.tile([128, ns * 255], f32, tag="in")
            nc.scalar.dma_start(
                out=in_t[:, :].rearrange("p (s c) -> p s c", c=255),
                in_=x_d[c0 : c0 + ncell, :].rearrange("(s p) c -> p s c", p=128),
            )

            # ---- cell-major transforms ----
            sig = sig_pool.tile([128, ns * 255], f32, tag="sig")
            nc.scalar.activation(sig[:, :], in_t[:, :], AF.Sigmoid)

            in_v = in_t[:, :].rearrange("p (s a c) -> p s a c", a=A, c=NUM_PRED)
            sig_v = sig[:, :].rearrange("p (s a c) -> p s a c", a=A, c=NUM_PRED)

            # exp(wh) = sigmoid(wh) / sigmoid(-wh)
            sgnw = sm_pool.tile([128, ns * 6], f32, tag="sgnw")
            nc.scalar.activation(
                sgnw[:, :].rearrange("p (s a k) -> p s a k", a=A, k=2),
                in_v[:, :, :, 2:4],
                AF.Sigmoid,
                scale=-1.0,
            )
            rec = sm_pool.tile([128, ns * 6], f32, tag="rec")
            nc.vector.reciprocal(rec[:, :], sgnw[:, :])
            t1 = sm_pool.tile([128, ns * 6], f32, tag="t1")
            nc.vector.tensor_tensor(
                t1[:, :].rearrange("p (s a k) -> p s a k", a=A, k=2),
                sig_v[:, :, :, 2:4],
                hanch_sb[:, : ns * 6].rearrange("p (s a k) -> p s a k", a=A, k=2),
                OP.mult,
            )
            halfwh = sm_pool.tile([128, ns * 6], f32, tag="halfwh")
            nc.vector.tensor_tensor(halfwh[:, :], t1[:, :], rec[:, :], OP.mult)

            # xy = sigmoid(xy)*stride + off*stride
            xy = sm_pool.tile([128, ns * 6], f32, tag="xy")
            nc.vector.scalar_tensor_tensor(
                xy[:, :].rearrange("p (s a k) -> p s a k", a=A, k=2),
                in0=sig_v[:, :, :, 0:2],
                scalar=stride_f,
                in1=offs_sb[:, t0 * 6 : (t0 + ns) * 6].rearrange(
                    "p (s a k) -> p s a k", a=A, k=2
                ),
                op0=OP.mult,
                op1=OP.add,
            )

            # per-subtile block layout [S_a0 | S_a1 | S_a2 | pad 16 | bb 12] so
            # anchor 2's transpose carries the bbox columns for free, landing
            # them on PE row group 3 (partitions 96..107)
            SW = A * C + 16 + 12  # 268
            S = s_pool.tile([128, ns * SW], f32, tag="S")

            # bbox cell-major -> S cols [240:252) per subtile: [a, 4] = x1 y1 x2 y2
            S_v = S[:, :].rearrange("p (s w) -> p s w", w=SW)
            bb_v = S[:, :].rearrange("p (s w) -> p s w", w=SW)[
                :, :, A * C + 16 : SW
            ].rearrange("p s (a k) -> p s a k", k=4)
            xy_v = xy[:, :].rearrange("p (s a k) -> p s a k", a=A, k=2)
            hw_v = halfwh[:, :].rearrange("p (s a k) -> p s a k", a=A, k=2)
            nc.vector.tensor_tensor(bb_v[:, :, :, 0:2], xy_v, hw_v, OP.subtract)
            nc.vector.tensor_tensor(bb_v[:, :, :, 2:4], xy_v, hw_v, OP.add)

            # class scores = sigmoid(cls) * sigmoid(obj), cell-major; obj is
            # broadcast along the class dim with a stride-0 AP (gpsimd: DVE
            # and ACT are the busy engines)
            nc.gpsimd.tensor_tensor(
                S[:, :]
                .rearrange("p (s w) -> p s w", w=SW)[:, :, 0 : A * C]
                .rearrange("p s (a c) -> p s a c", c=C),
                sig_v[:, :, :, 5:85],
                sig_v[:, :, :, 4:5].to_broadcast([128, ns, A, C]),
                OP.mult,
            )

            # output supertile, class-major
            outt = o_pool.tile([C, ncell * ROW], f32, tag="outt")
            ov = outt[:, :].rearrange("c (i e) -> c e i", e=ROW)

            bbt = stg_pool.tile([128, ncell], bc_dt, tag="bbt")

            for a in (2, 0, 1):  # anchor 2 first: it stages the bbox rows
                # transpose scores of anchor a -> [C, ncell]; anchor 2 also
                # carries the 12 bbox rows into partitions 96..107
                pw = C + 28 if a == 2 else C
                P = p_pool.tile([C + 28, ncell], f32, tag="P")
                for s in range(ns):
                    nc.tensor.transpose(
                        P[0:pw, s * 128 : (s + 1) * 128],
                        S_v[:, s, a * C : a * C + pw],
                        ident_sb[:, :],
                    )
                if a == 2:
                    # stage bbox rows to SBUF (rounds to fp32r) on row group 3
                    nc.vector.tensor_copy(bbt[96:108, :], P[96:108, :])
                    for base in BASES[1:ROW_GROUPS]:
                        nc.sync.dma_start(
                            out=bbt[base : base + 12, :], in_=bbt[96:108, :]
                        )
                mask = m_pool.tile([C, ncell], f32, tag="mask")
                nc.vector.tensor_scalar(mask[:, :], P[0:C, :], THRESH, None, OP.is_gt)
                # score -> column a*6+1 (strided 18)
                nc.vector.tensor_tensor(
                    ov[:, a * 6 + 1, :], P[0:C, :], mask[:, :], OP.mult
                )
                # cid = mask*(c+1) - 1 -> column a*6+0 (gpsimd, SBUF-only op)
                nc.gpsimd.tensor_scalar(
                    ov[:, a * 6 + 0, :],
                    mask[:, :],
                    cp1_sb[0:C, :],
                    -1.0,
                    OP.mult,
                    OP.add,
                )

                # bbox broadcast via one-hot selector matmuls (fp32r, one
                # pass) on 4 distinct PE row groups -> concurrent
                for half in range(2):
                    q = q_pool.tile([C, 2 * ncell], f32, tag="q")
                    for kk in range(2):
                        k = half * 2 + kk
                        j = a * 4 + k
                        base = BASES[j % ROW_GROUPS]
                        nc.tensor.matmul(
                            q[:, kk * ncell : (kk + 1) * ncell],
                            lhsT=sel_r[base : base + 12, j * C : (j + 1) * C],
                            rhs=bbt[base : base + 12, :],
                            start=True,
                            stop=True,
                            tile_position=(base, 0),
                        )
                    # drain both channels in one strided op; ACT takes 4 of
                    # 6 (it is 1.25x faster per element and less loaded)
                    dst = ov[:, a * 6 + 2 + half * 2 : a * 6 + 4 + half * 2, :]
                    src = q[:, :].rearrange("c (k i) -> c k i", k=2)
                    if (a, half) in ((0, 1), (1, 1)):
                        nc.vector.tensor_copy(dst, src)
                    else:
                        nc.scalar.copy(dst, src)

            # ---- store ----
            nc.sync.dma_start(
                out=out_d[:, c0 * ROW : (c0 + ncell) * ROW], in_=outt[:, :]
            )
            c0 += ncell

    nc.finalize()
    return nc


def make_consts(anchor, offset, stride_f, n_cells=HW_CELLS):
    """Pack [offs | hanch | ident | cp1 | sel] into one (128, F) f32 blob."""
    n_tiles = n_cells // 128
    off = np.asarray(offset, dtype=np.float32).reshape(-1, 2)[:n_cells] * stride_f
    offs6 = np.tile(off, (1, 3)).reshape(n_tiles, 128, 6)  # [t, p, j]
    offs_cols = np.ascontiguousarray(np.transpose(offs6, (1, 0, 2)).reshape(128, n_tiles * 6))
    a2 = np.asarray(anchor, dtype=np.float32).reshape(NUM_ANCHOR, 2)
    hanch = np.tile((a2 / 2.0).reshape(6), (128, 4)).astype(np.float32)
    ident = np.eye(128, dtype=np.float32)
    cp1 = np.broadcast_to(np.arange(1, 129, dtype=np.float32).reshape(128, 1), (128, 1))
    # one-hot selector for bbox channel j, placed on PE row group j%4 so the
    # four per-anchor broadcast matmuls can row-tile concurrently
    # one-hot selectors for bbox channel j, for every row-group mapping the
    # kernel might use (distinct bases never collide within a column block)
    sel128 = np.zeros((128, 12 * NUM_CLASSES), dtype=np.float32)
    bases = (96, 0, 32, 64)
    for rg in (1, 2, 4):
        for j in range(12):
            sel128[bases[j % rg] + j, j * NUM_CLASSES : (j + 1) * NUM_CLASSES] = 1.0
    blob = np.concatenate([offs_cols, hanch, ident, cp1, sel128], axis=1)
    return np.ascontiguousarray(blob.astype(np.float32))


def _host_prep(output, anchor, offset, stride):
    stride_f = float(stride)
    B = output.shape[0]
    x_all = np.ascontiguousarray(
        np.asarray(output, dtype=np.float32).reshape(B, HW_CELLS, NUM_ANCHOR * NUM_PRED)
    )
    consts = make_consts(anchor, offset, stride_f)
    return stride_f, x_all, consts


def kernel(output, anchor, offset, stride):
    from concourse.bass_utils import run_bass_kernel_spmd

    stride_f, x_all, consts = _host_prep(output, anchor, offset, stride)
    key = ("nc", stride_f)
    if key not in _CACHE:
        _CACHE[key] = _build(stride_f)
    nc = _CACHE[key]

    in_maps = [{"x": x_all[b], "consts": consts} for b in range(N_CORES)]
    res = run_bass_kernel_spmd(
        nc,
        in_maps,
        list(range(N_CORES)),
        tmpdir=os.environ.get("KERNEL_TRACE_DIR") or None,
    )
    global LAST_RESULT
    LAST_RESULT = res
    outs = [
        r["out"].reshape(NUM_CLASSES * HW_CELLS * NUM_ANCHOR, 6) for r in res.results
    ]
    return np.stack(outs, axis=0)


if __name__ == "__main__":
    rng = np.random.default_rng(0)
    out = rng.standard_normal((8, 80, 80, 255), dtype=np.float32)
    anchor = rng.uniform(10.0, 120.0, (1, 1, 3, 2)).astype(np.float32)
    gy, gx = np.meshgrid(np.arange(80, dtype=np.float32), np.arange(80, dtype=np.float32), indexing="ij")
    offset = np.stack([gx, gy], axis=-1).reshape(1, 80, 80, 1, 2)
    r = kernel(out, anchor, offset, 8)
    print(r.shape, r.dtype)


# revision 45
# speedup vs baseline: 2.0833x; 1.1620x over previous
"""Trainium2 Bass kernel for YOLO-style detection decode (nms_detection).

Computes, for input `output` (B=8, H=80, W=80, A*85=255):
  per (b, cell, anchor):  xy = (sigmoid(txy) + grid_off) * stride
                          wh = exp(twh) * anchor
                          bbox = [xy - wh/2, xy + wh/2]
                          p_c = sigmoid(cls_c) * sigmoid(obj)
  out (B, C*hw*A, 6) rows = [cid, score, x1, y1, x2, y2] where
  cid = c if p_c > 0.01 else -1, score = p_c if p_c > 0.01 else 0.

Sharding: pure data parallel over batch, one batch element per NeuronCore.

Per-core layout strategy (output is 37 MB/core -> write-bandwidth bound):
  - cells are processed in supertiles of 512 (4 subtiles of 128 = partition dim)
  - class scores are transposed (TensorE) to class-major (80 partitions) so the
    per-class output block (hw, A, 6) is DMA'd with 9 KB contiguous segments
  - bbox (class-independent) is broadcast to all 80 class partitions with
    one-hot "selector" matmuls (K=12) on the otherwise idle TensorE
  - exp(x) is computed as sigmoid(x)/sigmoid(-x) so the ScalarE activation
    table never leaves the sigmoid set (a table switch costs ~2.7us).
"""

import sys
import os
from contextlib import ExitStack

if "/opt/trn_rl_repo" not in sys.path:
    sys.path.insert(0, "/opt/trn_rl_repo")

import numpy as np

NUM_CLASSES = 80
NUM_ANCHOR = 3
NUM_PRED = 85
HW_CELLS = 6400
THRESH = 0.01
N_CORES = 8
ROW = 6 * NUM_ANCHOR  # f32 per cell per class in the output (18)

_CACHE = {}
LAST_RESULT = None  # BassKernelResults of the most recent kernel() call


# Row groups used for the broadcast matmuls (1, 2, or 4). Multi-group runs
# the per-anchor broadcasts concurrently on distinct PE row groups.
ROW_GROUPS = int(os.environ.get("KERNEL_ROW_GROUPS", "1"))
BASES = (96, 0, 32, 64)
BCAST_DTYPE = os.environ.get("KERNEL_BCAST_DTYPE", "f32r")  # f32r | f32


def _build(stride_f: float, n_cells: int = HW_CELLS):
    import concourse.bass as bass  # noqa: F401
    import concourse.bacc as bacc
    import concourse.tile as tile
    from concourse import mybir

    f32 = mybir.dt.float32
    f32r = mybir.dt.float32r
    AF = mybir.ActivationFunctionType
    OP = mybir.AluOpType

    C = NUM_CLASSES
    A = NUM_ANCHOR

    n_tiles = n_cells // 128
    CONST_F = n_tiles * 6 + 24 + 128 + 1 + 12 * C  # offs | hanch | ident | cp1 | sel
    OFF_HANCH = n_tiles * 6
    OFF_IDENT = OFF_HANCH + 24
    OFF_CP1 = OFF_IDENT + 128
    OFF_SEL = OFF_CP1 + 1

    nc = bacc.Bacc("TRN2", target_bir_lowering=False, debug=False)
    x_d = nc.declare_dram_parameter("x", [n_cells, A * NUM_PRED], f32, isOutput=False)
    const_d = nc.declare_dram_parameter("consts", [128, CONST_F], f32, isOutput=False)
    out_d = nc.declare_dram_parameter("out", [C, n_cells * ROW], f32, isOutput=True)

    # supertile = up to 4 subtiles of 128 cells
    st_sizes = []
    left = n_cells
    while left > 0:
        take = min(512, left)
        assert take % 128 == 0
        st_sizes.append(take)
        left -= take

    with ExitStack() as ctx:
        tc = ctx.enter_context(tile.TileContext(nc))
        cpool = ctx.enter_context(tc.tile_pool(name="const", bufs=1))
        in_pool = ctx.enter_context(tc.tile_pool(name="inp", bufs=3))
        sig_pool = ctx.enter_context(tc.tile_pool(name="sig", bufs=2))
        sm_pool = ctx.enter_context(tc.tile_pool(name="small", bufs=2))
        s_pool = ctx.enter_context(tc.tile_pool(name="scls", bufs=2))
        m_pool = ctx.enter_context(tc.tile_pool(name="mask", bufs=2))
        stg_pool = ctx.enter_context(tc.tile_pool(name="stage", bufs=2))
        o_pool = ctx.enter_context(tc.tile_pool(name="outt", bufs=2))
        p_pool = ctx.enter_context(tc.tile_pool(name="ppsum", bufs=2, space="PSUM"))
        q_pool = ctx.enter_context(tc.tile_pool(name="qpsum", bufs=2, space="PSUM"))

        # ---- constants (one DMA -> one sem lane) ----
        const_sb = cpool.tile([128, CONST_F], f32, tag="consts")
        nc.scalar.dma_start(out=const_sb[:, :], in_=const_d[:, :])
        offs_sb = const_sb[:, 0:OFF_HANCH]
        hanch_sb = const_sb[:, OFF_HANCH:OFF_IDENT]
        ident_sb = const_sb[:, OFF_IDENT:OFF_CP1]
        cp1_sb = const_sb[:, OFF_CP1:OFF_SEL]
        sel_sb = const_sb[:, OFF_SEL:CONST_F]
        bc_dt = f32r if BCAST_DTYPE == "f32r" else f32
        sel_r = cpool.tile([128, 12 * C], bc_dt, tag="selr")
        nc.vector.tensor_copy(sel_r[:, :], sel_sb)

        # ---- warm-up: let each engine observe the const DMA once, so no
        # later instruction needs more than one sync-wait (ISA limit) ----
        warm = cpool.tile([128, 4], f32, tag="warm")
        nc.vector.tensor_copy(warm[0:1, 0:1], const_sb[0:1, 0:1])
        nc.scalar.copy(warm[0:1, 1:2], const_sb[0:1, 0:1])
        nc.gpsimd.tensor_copy(warm[0:1, 2:3], const_sb[0:1, 0:1])
        wq = p_pool.tile([128, 128], f32, tag="P")
        nc.tensor.transpose(wq[:, :], ident_sb, ident_sb)

        c0 = 0
        for st, ncell in enumerate(st_sizes):
            ns = ncell // 128  # subtiles
            t0 = c0 // 128

            # ---- load input supertile: [p, s, 255] ----
            in_t = in_pool.tile([128, ns * 255], f32, tag="in")
            nc.scalar.dma_start(
                out=in_t[:, :].rearrange("p (s c) -> p s c", c=255),
                in_=x_d[c0 : c0 + ncell, :].rearrange("(s p) c -> p s c", p=128),
            )

            # ---- cell-major transforms ----
            sig = sig_pool.tile([128, ns * 255], f32, tag="sig")
            nc.scalar.activation(sig[:, :], in_t[:, :], AF.Sigmoid)

            in_v = in_t[:, :].rearrange("p (s a c) -> p s a c", a=A, c=NUM_PRED)
            sig_v = sig[:, :].rearrange("p (s a c) -> p s a c", a=A, c=NUM_PRED)

            # exp(wh) = sigmoid(wh) / sigmoid(-wh)
            sgnw = sm_pool.tile([128, ns * 6], f32, tag="sgnw")
            nc.scalar.activation(
                sgnw[:, :].rearrange("p (s a k) -> p s a k", a=A, k=2),
                in_v[:, :, :, 2:4],
                AF.Sigmoid,
                scale=-1.0,
            )
            rec = sm_pool.tile([128, ns * 6], f32, tag="rec")
            nc.vector.reciprocal(rec[:, :], sgnw[:, :])
            t1 = sm_pool.tile([128, ns * 6], f32, tag="t1")
            nc.vector.tensor_tensor(
                t1[:, :].rearrange("p (s a k) -> p s a k", a=A, k=2),
                sig_v[:, :, :, 2:4],
                hanch_sb[:, : ns * 6].rearrange("p (s a k) -> p s a k", a=A, k=2),
                OP.mult,
            )
            halfwh = sm_pool.tile([128, ns * 6], f32, tag="halfwh")
            nc.vector.tensor_tensor(halfwh[:, :], t1[:, :], rec[:, :], OP.mult)

            # xy = sigmoid(xy)*stride + off*stride
            xy = sm_pool.tile([128, ns * 6], f32, tag="xy")
            nc.vector.scalar_tensor_tensor(
                xy[:, :].rearrange("p (s a k) -> p s a k", a=A, k=2),
                in0=sig_v[:, :, :, 0:2],
                scalar=stride_f,
                in1=offs_sb[:, t0 * 6 : (t0 + ns) * 6].rearrange(
                    "p (s a k) -> p s a k", a=A, k=2
                ),
                op0=OP.mult,
                op1=OP.add,
            )

            # per-subtile block layout [S_a0 | S_a1 | S_a2 | pad 16 | bb 12] so
            # anchor 2's transpose carries the bbox columns for free, landing
            # them on PE row group 3 (partitions 96..107)
            SW = A * C + 16 + 12  # 268
            S = s_pool.tile([128, ns * SW], f32, tag="S")

            # bbox cell-major -> S cols [240:252) per subtile: [a, 4] = x1 y1 x2 y2
            S_v = S[:, :].rearrange("p (s w) -> p s w", w=SW)
            bb_v = S[:, :].rearrange("p (s w) -> p s w", w=SW)[
                :, :, A * C + 16 : SW
            ].rearrange("p s (a k) -> p s a k", k=4)
            xy_v = xy[:, :].rearrange("p (s a k) -> p s a k", a=A, k=2)
            hw_v = halfwh[:, :].rearrange("p (s a k) -> p s a k", a=A, k=2)
            nc.vector.tensor_tensor(bb_v[:, :, :, 0:2], xy_v, hw_v, OP.subtract)
            nc.vector.tensor_tensor(bb_v[:, :, :, 2:4], xy_v, hw_v, OP.add)

            # class scores = sigmoid(cls) * sigmoid(obj), cell-major; obj is
            # broadcast along the class dim with a stride-0 AP (gpsimd: DVE
            # and ACT are the busy engines)
            nc.gpsimd.tensor_tensor(
                S[:, :]
                .rearrange("p (s w) -> p s w", w=SW)[:, :, 0 : A * C]
                .rearrange("p s (a c) -> p s a c", c=C),
                sig_v[:, :, :, 5:85],
                sig_v[:, :, :, 4:5].to_broadcast([128, ns, A, C]),
                OP.mult,
            )

            # output supertile, class-major
            outt = o_pool.tile([C, ncell * ROW], f32, tag="outt")
            ov = outt[:, :].rearrange("c (i e) -> c e i", e=ROW)

            bbt = stg_pool.tile([128, ncell], bc_dt, tag="bbt")

            for a in (2, 0, 1):  # anchor 2 first: it stages the bbox rows
                # transpose scores of anchor a -> [C, ncell]; anchor 2 also
                # carries the 12 bbox rows into partitions 96..107
                pw = C + 28 if a == 2 else C
                P = p_pool.tile([C + 28, ncell], f32, tag="P")
                for s in range(ns):
                    nc.tensor.transpose(
                        P[0:pw, s * 128 : (s + 1) * 128],
                        S_v[:, s, a * C : a * C + pw],
                        ident_sb[:, :],
                    )
                if a == 2:
                    # stage bbox rows to SBUF (rounds to fp32r) on row group 3
                    nc.vector.tensor_copy(bbt[96:108, :], P[96:108, :])
                    for base in BASES[1:ROW_GROUPS]:
                        nc.sync.dma_start(
                            out=bbt[base : base + 12, :], in_=bbt[96:108, :]
                        )
                mask = m_pool.tile([C, ncell], f32, tag="mask")
                nc.vector.tensor_scalar(mask[:, :], P[0:C, :], THRESH, None, OP.is_gt)
                # score -> column a*6+1 (strided 18)
                nc.vector.tensor_tensor(
                    ov[:, a * 6 + 1, :], P[0:C, :], mask[:, :], OP.mult
                )
                # cid = mask*(c+1) - 1 -> column a*6+0 (gpsimd, SBUF-only op)
                nc.gpsimd.tensor_scalar(
                    ov[:, a * 6 + 0, :],
                    mask[:, :],
                    cp1_sb[0:C, :],
                    -1.0,
                    OP.mult,
                    OP.add,
                )

                # bbox broadcast via one-hot selector matmuls (fp32r, one
                # pass) on 4 distinct PE row groups -> concurrent
                for half in range(2):
                    q = q_pool.tile([C, 2 * ncell], f32, tag="q")
                    for kk in range(2):
                        k = half * 2 + kk
                        j = a * 4 + k
                        base = BASES[j % ROW_GROUPS]
                        nc.tensor.matmul(
                            q[:, kk * ncell : (kk + 1) * ncell],
                            lhsT=sel_r[base : base + 12, j * C : (j + 1) * C],
                            rhs=bbt[base : base + 12, :],
                            start=True,
                            stop=True,
                            tile_position=(base, 0),
                        )
                    # drain both channels in one strided op; ACT takes 4 of
                    # 6 (it is 1.25x faster per element and less loaded)
                    dst = ov[:, a * 6 + 2 + half * 2 : a * 6 + 4 + half * 2, :]
                    src = q[:, :].rearrange("c (k i) -> c k i", k=2)
                    if (a, half) in ((0, 1), (1, 1)):
                        nc.vector.tensor_copy(dst, src)
                    else:
                        nc.scalar.copy(dst, src)

            # ---- store ----
            nc.sync.dma_start(
                out=out_d[:, c0 * ROW : (c0 + ncell) * ROW], in_=outt[:, :]
            )
            c0 += ncell

    nc.finalize()
    return nc


def make_consts(anchor, offset, stride_f, n_cells=HW_CELLS):
    """Pack [offs | hanch | ident | cp1 | sel] into one (128, F) f32 blob."""
    n_tiles = n_cells // 128
    off = np.asarray(offset, dtype=np.float32).reshape(-1, 2)[:n_cells] * stride_f
    offs6 = np.tile(off, (1, 3)).reshape(n_tiles, 128, 6)  # [t, p, j]
    offs_cols = np.ascontiguousarray(np.transpose(offs6, (1, 0, 2)).reshape(128, n_tiles * 6))
    a2 = np.asarray(anchor, dtype=np.float32).reshape(NUM_ANCHOR, 2)
    hanch = np.tile((a2 / 2.0).reshape(6), (128, 4)).astype(np.float32)
    ident = np.eye(128, dtype=np.float32)
    cp1 = np.broadcast_to(np.arange(1, 129, dtype=np.float32).reshape(128, 1), (128, 1))
    # one-hot selector for bbox channel j, placed on PE row group j%4 so the
    # four per-anchor broadcast matmuls can row-tile concurrently
    # one-hot selectors for bbox channel j, for every row-group mapping the
    # kernel might use (distinct bases never collide within a column block)
    sel128 = np.zeros((128, 12 * NUM_CLASSES), dtype=np.float32)
    bases = (96, 0, 32, 64)
    for rg in (1, 2, 4):
        for j in range(12):
            sel128[bases[j % rg] + j, j * NUM_CLASSES : (j + 1) * NUM_CLASSES] = 1.0
    blob = np.concatenate([offs_cols, hanch, ident, cp1, sel128], axis=1)
    return np.ascontiguousarray(blob.astype(np.float32))


def _host_prep(output, anchor, offset, stride):
    stride_f = float(stride)
    B = output.shape[0]
    x_all = np.ascontiguousarray(
        np.asarray(output, dtype=np.float32).reshape(B, HW_CELLS, NUM_ANCHOR * NUM_PRED)
    )
    consts = make_consts(anchor, offset, stride_f)
    return stride_f, x_all, consts


def kernel(output, anchor, offset, stride):
    from concourse.bass_utils import run_bass_kernel_spmd

    stride_f, x_all, consts = _host_prep(output, anchor, offset, stride)
    key = ("nc", stride_f)
    if key not in _CACHE:
        _CACHE[key] = _build(stride_f)
    nc = _CACHE[key]

    in_maps = [{"x": x_all[b], "consts": consts} for b in range(N_CORES)]
    res = run_bass_kernel_spmd(
        nc,
        in_maps,
        list(range(N_CORES)),
        tmpdir=os.environ.get("KERNEL_TRACE_DIR") or None,
    )
    global LAST_RESULT
    LAST_RESULT = res
    outs = [
        r["out"].reshape(NUM_CLASSES * HW_CELLS * NUM_ANCHOR, 6) for r in res.results
    ]
    return np.stack(outs, axis=0)


if __name__ == "__main__":
    rng = np.random.default_rng(0)
    out = rng.standard_normal((8, 80, 80, 255), dtype=np.float32)
    anchor = rng.uniform(10.0, 120.0, (1, 1, 3, 2)).astype(np.float32)
    gy, gx = np.meshgrid(np.arange(80, dtype=np.float32), np.arange(80, dtype=np.float32), indexing="ij")
    offset = np.stack([gx, gy], axis=-1).reshape(1, 80, 80, 1, 2)
    r = kernel(out, anchor, offset, 8)
    print(r.shape, r.dtype)
